# revision 1
# baseline (speedup 1.0000x reference)
"""RWKV-v4 block (time-mix WKV attention + channel-mix GLU) on 8 Trainium2
NeuronCores, data-parallel over batch B.

Layouts per core (B_local=4, T=1024, C=512, H=2048):
  - layout A: [t(128p), n(8), c(512)]  -- LayerNorm (per-partition stats),
    residual adds, final store.
  - layout B: [c(128p) x 4 chunks, t(1024)] -- mixing, WKV scan (along free
    dim), k/v/r matmuls.
  - A->B via bf16 DMA transpose through a DRAM bounce; B->A avoided by
    running Wo/cWv/cWr matmuls with the *activation* as the stationary
    operand (out = act.T @ W.T lands in layout A).

WKV: with per-channel M = max_t k, e=exp(k-M), the recurrence
  P_t = d*P_{t-1} + e_t*v_t,  Q_t = d*Q_{t-1} + e_t   (d = exp(-exp(decay)))
  y_t = (P_{t-1} + exp(u)*e_t*v_t) / (Q_{t-1} + exp(u)*e_t)
matches the reference's max-tracking scan exactly (the M scaling cancels in
the ratio).  Runs as two native tensor_tensor_scan ops per 128-channel chunk.
"""

import numpy as np
import ml_dtypes
from contextlib import ExitStack

import concourse.bass as bass
import concourse.tile as tile
from concourse import bacc, mybir

B, T, C = 32, 1024, 512
H = 4 * C
NCORES = 8
BL = B // NCORES  # batches per core
NT = T // 128     # 8 t-subtiles per batch
CC = C // 128     # 4 channel chunks
HC = H // 128     # 16 hidden chunks

F32 = mybir.dt.float32
BF16 = mybir.dt.bfloat16
AX = mybir.AxisListType
OP = mybir.AluOpType
AF = mybir.ActivationFunctionType


PHASE_LOG = []


def _emit(nc, tc, ctx, io, bl):
    """Emit the whole per-core program."""
    PHASE_LOG.clear()

    def mark(label):
        PHASE_LOG.append((nc.next_id(), label))

    x_d = io["x"].ap()
    y_d = io["y"].ap()

    def col(name, c0):  # [128,1] slice of a [N] dram vector
        return io[name].ap()[c0 * 128:(c0 + 1) * 128].rearrange(
            "(c one) -> c one", one=1)

    sb = ctx.enter_context(tc.tile_pool(name="sb", bufs=1))
    ps = ctx.enter_context(tc.tile_pool(name="ps", bufs=8, space="PSUM"))
    dramp = ctx.enter_context(tc.tile_pool(name="dram", bufs=2, space="DRAM"))

    # ---- constants / weights (resident) ----
    def load_w(name, rows, cols):
        tiles = []
        for i in range(rows // 128):
            t_ = sb.tile([128, cols], BF16, tag=f"w_{name}_{i}")
            nc.sync.dma_start(t_[:], io[name].ap()[i * 128:(i + 1) * 128, :])
            tiles.append(t_)
        return tiles

    wkT = load_w("wkT", C, C)
    wvT = load_w("wvT", C, C)
    wrT = load_w("wrT", C, C)
    woT = load_w("woT", C, C)
    cwkT = load_w("cwkT", C, H)
    cwvT = load_w("cwvT", H, C)
    cwrT = load_w("cwrT", C, C)

    def vec4(name):
        ts_ = []
        for i in range(CC):
            t_ = sb.tile([128, 1], F32, tag=f"v_{name}_{i}")
            nc.sync.dma_start(t_[:], col(name, i))
            ts_.append(t_)
        return ts_

    delta_c = vec4("delta")
    expu_c = vec4("expu")
    vb_c = vec4("vb")
    rb_c = vec4("rb")

    def vec4_m1(name):
        # coef - 1 (mix runs as o = xn + (coef-1)*d so every operand reads
        # at an aligned offset; the t-shift lives only inside d)
        ts_ = []
        for i in range(CC):
            t_ = sb.tile([128, 1], F32, tag=f"vm_{name}_{i}")
            nc.sync.dma_start(t_[:], col(name, i))
            nc.vector.tensor_scalar_add(t_[:], t_[:], -1.0)
            ts_.append(t_)
        return ts_

    tmk_c = vec4_m1("tmk")
    tmv_c = vec4_m1("tmv")
    tmr_c = vec4_m1("tmr")
    cmk_c = vec4_m1("cmk")
    cmr_c = vec4_m1("cmr")
    kkb_c = []
    for i in range(HC):
        t_ = sb.tile([128, 1], F32, tag=f"v_kkb_{i}")
        nc.sync.dma_start(t_[:], col("kkb", i))
        kkb_c.append(t_)

    eps_t = sb.tile([128, 1], F32, tag="eps")
    nc.vector.memset(eps_t[:], 1e-5)
    zrow = sb.tile([32, C], BF16, tag="zrow")
    nc.vector.memset(zrow[:], 0.0)


    # ---- per-batch pipeline ----
    xa_pool = ctx.enter_context(tc.tile_pool(name="xa", bufs=1))
    x1_pool = ctx.enter_context(tc.tile_pool(name="x1", bufs=1))
    lnp = ctx.enter_context(tc.tile_pool(name="ln", bufs=1))
    bp = ctx.enter_context(tc.tile_pool(name="bp", bufs=1))
    wkvp = ctx.enter_context(tc.tile_pool(name="wkv", bufs=1))
    srwp = ctx.enter_context(tc.tile_pool(name="srw", bufs=1))
    cmp_ = ctx.enter_context(tc.tile_pool(name="cm", bufs=1))
    outp = ctx.enter_context(tc.tile_pool(name="out", bufs=2))

    def layer_norm(src_tile, which):
        """src [128, NT, 512] fp32 (layout A) -> 4x [128, T] bf16 layout-B
        tiles of the *raw* normalized activations (g/b folded into weights
        downstream)."""
        sums = lnp.tile([128, NT], F32, tag="sums")
        sqs = lnp.tile([128, NT], F32, tag="sqs")
        scr = lnp.tile([128, 512], BF16, tag="scr")
        # all Copies then all Squares: ACT table switches are ~1.3us each
        for n in range(NT):
            nc.scalar.activation(scr[:], src_tile[:, n, :], AF.Copy,
                                 accum_out=sums[:, n:n + 1])
        for n in range(NT):
            nc.scalar.activation(scr[:], src_tile[:, n, :], AF.Square,
                                 accum_out=sqs[:, n:n + 1])
        mean = lnp.tile([128, NT], F32, tag="mean")
        nc.vector.tensor_scalar_mul(mean[:], sums[:], 1.0 / C)
        msq = lnp.tile([128, NT], F32, tag="msq")
        nc.scalar.activation(msq[:], mean[:], AF.Square)
        var = lnp.tile([128, NT], F32, tag="var")
        nc.vector.scalar_tensor_tensor(var[:], sqs[:], 1.0 / C, msq[:],
                                       op0=OP.mult, op1=OP.subtract)
        sqv = lnp.tile([128, NT], F32, tag="sqv")
        nc.scalar.activation(sqv[:], var[:], AF.Sqrt, bias=eps_t[:])
        rstd = lnp.tile([128, NT], F32, tag="rstd")
        nc.vector.reciprocal(rstd[:], sqv[:])
        xn = lnp.tile([128, NT, 512], BF16, tag="xn")
        for n in range(NT):
            nc.vector.tensor_scalar(xn[:, n, :], src_tile[:, n, :],
                                    mean[:, n:n + 1], rstd[:, n:n + 1],
                                    op0=OP.subtract, op1=OP.mult)
        # bounce through DRAM with a zero row at t=0, reload transposed into
        # layout B; the zero lands in column 0 so the time-shift is a plain
        # offset view (transpose dest must stay column-aligned on HW)
        xnd = dramp.tile([T + 32, C], BF16, tag="xnd")
        nc.sync.dma_start(xnd[0:32, :], zrow[:])
        nc.sync.dma_start(xnd[32:T + 32].rearrange("(n p) c -> p n c", p=128),
                          xn[:])
        xnB, dB = [], []
        for cc in range(CC):
            t_ = bp.tile([128, T + 32], BF16, tag=f"xnB_{cc}")
            nc.sync.dma_start_transpose(t_[:],
                                        xnd[:, cc * 128:(cc + 1) * 128])
            xnB.append(t_)
            # shared per-cc delta (xn_t - xn_{t-1}) reused by all mix branches
            d = bp.tile([128, T], BF16, tag=f"mixd_{cc}")
            nc.gpsimd.tensor_tensor(d[:], t_[:, 32:T + 32], t_[:, 31:T + 31],
                                    op=OP.subtract)
            dB.append(d)
        return xnB, dB

    def mix(xnB, dB, coefm1_c, slot, cc):
        """xk = coef*xn + (1-coef)*xx = xn + (coef-1)*d, all reads aligned."""
        o = bp.tile([128, T], BF16, tag=f"mix_{slot}_{cc}")
        nc.vector.scalar_tensor_tensor(o[:], dB[cc][:], coefm1_c[cc][:],
                                       xnB[cc][:, 32:T + 32],
                                       op0=OP.mult, op1=OP.add)
        return o

    for b in range(bl):
        xb = x_d[b].rearrange("(n p) c -> p n c", p=128)
        yb = y_d[b].rearrange("(n p) c -> p n c", p=128)
        xa = xa_pool.tile([128, NT, 512], F32, tag="xa")
        nc.sync.dma_start(xa[:], xb)

        # ---------- time mix ----------
        mark(f"b{b}.ln1")
        xnB, dB = layer_norm(xa, "ln1")
        mark(f"b{b}.mix1")
        xk = [mix(xnB, dB, tmk_c, "k", cc) for cc in range(CC)]
        xv = [mix(xnB, dB, tmv_c, "v", cc) for cc in range(CC)]
        xr = [mix(xnB, dB, tmr_c, "r", cc) for cc in range(CC)]

        srw = []
        for hh in range(CC):
            mark(f"b{b}.wkv{hh}")
            # k/v/r for this 128-channel output chunk, t in halves.
            # ci outer / th inner: consecutive matmuls share the stationary.
            def mm_pair(wT, xs, tag):
                halves = [ps.tile([128, 512], F32, tag="ps", name=f"ps_{tag}{th}")
                          for th in range(2)]
                for ci in range(CC):
                    for th in range(2):
                        nc.tensor.matmul(
                            halves[th][:], wT[ci][:, hh * 128:(hh + 1) * 128],
                            xs[ci][:, th * 512:(th + 1) * 512],
                            start=(ci == 0), stop=(ci == CC - 1))
                return halves

            # order k, r, v: k's psum is freed by a dep-free ACT copy, r by the
            # dep-free sigmoid, and by the time v lands its consumer (ev,
            # which needs e) is ready -- so the PSUM ring never stalls PE.
            k_ps = mm_pair(wkT, xk, "k")
            m2 = wkvp.tile([128, 2], F32, tag="m2")
            ksb = wkvp.tile([128, T], BF16, tag="ksb")
            for th in range(2):
                nc.vector.tensor_reduce(m2[:, th:th + 1], k_ps[th][:],
                                        axis=AX.X, op=OP.max)
                nc.scalar.activation(ksb[:, th * 512:(th + 1) * 512],
                                     k_ps[th][:], AF.Copy)
            r_ps = mm_pair(wrT, xr, "r")
            sig = wkvp.tile([128, T], BF16, tag="sig")
            for th in range(2):
                nc.scalar.activation(sig[:, th * 512:(th + 1) * 512],
                                     r_ps[th][:], AF.Sigmoid, bias=rb_c[hh][:])
            v_ps = mm_pair(wvT, xv, "v")
            mneg = wkvp.tile([128, 1], F32, tag="mneg")
            nc.vector.tensor_reduce(mneg[:], m2[:], axis=AX.X, op=OP.max,
                                    negate=True)
            e = wkvp.tile([128, T], F32, tag="e")
            nc.scalar.activation(e[:], ksb[:], AF.Exp, bias=mneg[:])
            ev = wkvp.tile([128, T], F32, tag="ev")
            for th in range(2):
                sl = slice(th * 512, (th + 1) * 512)
                nc.vector.scalar_tensor_tensor(ev[:, sl], v_ps[th][:],
                                               vb_c[hh][:], e[:, sl],
                                               op0=OP.add, op1=OP.mult)
            Pb = wkvp.tile([128, T + 1], F32, tag="Pb")
            Qb = wkvp.tile([128, T + 1], F32, tag="Qb")
            nc.vector.memset(Pb[:, 0:1], 0.0)
            nc.vector.memset(Qb[:, 0:1], 0.0)
            db = delta_c[hh][:].to_broadcast((128, T))
            nc.vector.tensor_tensor_scan(Pb[:, 1:T + 1], db, ev[:],
                                         0.0, op0=OP.mult, op1=OP.add)
            nc.vector.tensor_tensor_scan(Qb[:, 1:T + 1], db, e[:],
                                         0.0, op0=OP.mult, op1=OP.add)
            # N over ev, D over e (in place)
            nc.vector.scalar_tensor_tensor(ev[:], ev[:], expu_c[hh][:],
                                           Pb[:, 0:T], op0=OP.mult, op1=OP.add)
            nc.vector.scalar_tensor_tensor(e[:], e[:], expu_c[hh][:],
                                           Qb[:, 0:T], op0=OP.mult, op1=OP.add)
            rec = Qb[:, 0:T]  # Qshift already consumed by the D stt above
            nc.vector.reciprocal_approx_fast(rec, e[:])
            nc.vector.tensor_tensor(ev[:], ev[:], rec, op=OP.mult)
            s_ = srwp.tile([128, T], BF16, tag=f"srw_{hh}")
            nc.vector.tensor_tensor(s_[:], ev[:], sig[:], op=OP.mult)
            srw.append(s_)

        mark(f"b{b}.wo")
        # Wo (activation-stationary) + residual, layout A
        x1 = x1_pool.tile([128, NT, 512], F32, tag="x1")
        for n in range(NT):
            p_ = ps.tile([128, 512], F32, tag="ps")
            for cc in range(CC):
                nc.tensor.matmul(p_[:], srw[cc][:, n * 128:(n + 1) * 128],
                                 woT[cc][:], start=(cc == 0), stop=(cc == CC - 1))
            nc.vector.tensor_tensor(x1[:, n, :], xa[:, n, :], p_[:], op=OP.add)

        # ---------- channel mix ----------
        mark(f"b{b}.ln2")
        xn2B, d2B = layer_norm(x1, "ln2")
        xk2 = [mix(xn2B, d2B, cmk_c, "k", cc) for cc in range(CC)]
        xr2 = [mix(xn2B, d2B, cmr_c, "r", cc) for cc in range(CC)]

        for th in range(2):
            mark(f"b{b}.cm{th}")
            tsl = slice(th * 512, (th + 1) * 512)
            kk2 = cmp_.tile([128, HC, 512], BF16, tag="kk2")
            for hh in range(HC):
                p_ = ps.tile([128, 512], F32, tag="ps")
                for ci in range(CC):
                    nc.tensor.matmul(p_[:], cwkT[ci][:, hh * 128:(hh + 1) * 128],
                                     xk2[ci][:, tsl],
                                     start=(ci == 0), stop=(ci == CC - 1))
                nc.scalar.activation(kk2[:, hh, :], p_[:], AF.Relu,
                                     bias=kkb_c[hh][:])
                nc.scalar.activation(kk2[:, hh, :], kk2[:, hh, :], AF.Square)
            for nn in range(4):
                n = th * 4 + nn
                rp = ps.tile([128, 512], F32, tag="ps")
                for ci in range(CC):
                    nc.tensor.matmul(rp[:], xr2[ci][:, n * 128:(n + 1) * 128],
                                     cwrT[ci][:], start=(ci == 0),
                                     stop=(ci == CC - 1))
                sig2 = outp.tile([128, 512], BF16, tag="sig2")
                nc.scalar.activation(sig2[:], rp[:], AF.Sigmoid)
                kvp = ps.tile([128, 512], F32, tag="ps")
                for hh in range(HC):
                    nc.tensor.matmul(kvp[:], kk2[:, hh, nn * 128:(nn + 1) * 128],
                                     cwvT[hh][:], start=(hh == 0),
                                     stop=(hh == HC - 1))
                t2 = outp.tile([128, 512], F32, tag="t2")
                nc.vector.tensor_tensor(t2[:], kvp[:], sig2[:], op=OP.mult)
                nc.gpsimd.tensor_tensor(t2[:], t2[:], x1[:, n, :], op=OP.add)
                nc.sync.dma_start(yb[:, n, :], t2[:])


def build_program(bl=BL):
    nc = bacc.Bacc("TRN2", target_bir_lowering=False, debug=False,
                   num_devices=NCORES)
    io = {}
    io["x"] = nc.dram_tensor("x", [bl, T, C], F32, kind="ExternalInput")
    io["y"] = nc.dram_tensor("y", [bl, T, C], F32, kind="ExternalOutput")
    for nm, shp in [("wkT", [C, C]), ("wvT", [C, C]), ("wrT", [C, C]),
                    ("woT", [C, C]), ("cwkT", [C, H]), ("cwvT", [H, C]),
                    ("cwrT", [C, C])]:
        io[nm] = nc.dram_tensor(nm, shp, BF16, kind="ExternalInput")
    for nm, n in [("delta", C), ("expu", C), ("tmk", C), ("tmv", C),
                  ("tmr", C), ("cmk", C), ("cmr", C), ("vb", C), ("rb", C),
                  ("kkb", H)]:
        io[nm] = nc.dram_tensor(nm, [n], F32, kind="ExternalInput")

    with tile.TileContext(nc) as tc:
        with ExitStack() as ctx:
            _emit(nc, tc, ctx, io, bl)
    nc.compile()
    return nc


def host_params(inputs):
    """Host-side parameter prep (O(C^2) only): transposes, LN gamma folding,
    bias projections, scan constants."""
    f32 = np.float32
    g1 = np.asarray(inputs["ln1_g"], f32)
    b1 = np.asarray(inputs["ln1_b"], f32)
    g2 = np.asarray(inputs["ln2_g"], f32)
    b2 = np.asarray(inputs["ln2_b"], f32)
    Wk = np.asarray(inputs["Wk"], f32)
    Wv = np.asarray(inputs["Wv"], f32)
    Wr = np.asarray(inputs["Wr"], f32)
    Wo = np.asarray(inputs["Wo"], f32)
    cWk = np.asarray(inputs["cWk"], f32)
    cWr = np.asarray(inputs["cWr"], f32)
    cWv = np.asarray(inputs["cWv"], f32)

    # r2 bias (cWr @ b2) would be a per-free-dim bias in the layout-A sigmoid;
    # only the zero case is supported (true for this model's init).
    r2b = cWr @ b2
    assert np.allclose(r2b, 0.0, atol=1e-30), "nonzero ln2_b not supported"

    bf = ml_dtypes.bfloat16
    p = {
        "wkT": np.ascontiguousarray((Wk.T * g1[:, None]).astype(bf)),
        "wvT": np.ascontiguousarray((Wv.T * g1[:, None]).astype(bf)),
        "wrT": np.ascontiguousarray((Wr.T * g1[:, None]).astype(bf)),
        "woT": np.ascontiguousarray(Wo.T.astype(bf)),
        "cwkT": np.ascontiguousarray((cWk.T * g2[:, None]).astype(bf)),
        "cwvT": np.ascontiguousarray(cWv.T.astype(bf)),
        "cwrT": np.ascontiguousarray((cWr.T * g2[:, None]).astype(bf)),
        "delta": np.exp(-np.exp(np.asarray(inputs["time_decay"], f32))),
        "expu": np.exp(np.asarray(inputs["time_first"], f32)),
        "tmk": np.asarray(inputs["tm_k"], f32),
        "tmv": np.asarray(inputs["tm_v"], f32),
        "tmr": np.asarray(inputs["tm_r"], f32),
        "cmk": np.asarray(inputs["cm_k"], f32),
        "cmr": np.asarray(inputs["cm_r"], f32),
        "vb": (Wv @ b1).astype(f32),
        "rb": (Wr @ b1).astype(f32),
        "kkb": (cWk @ b2).astype(f32),
    }
    return p


_CACHE = {}


def kernel(**inputs):
    from concourse.bass_utils import run_bass_kernel_spmd

    if "nc" not in _CACHE:
        _CACHE["nc"] = build_program(BL)
    nc = _CACHE["nc"]

    p = host_params(inputs)
    x = np.asarray(inputs["x"], np.float32)
    in_maps = []
    for c in range(NCORES):
        m = dict(p)
        m["x"] = np.ascontiguousarray(x[c * BL:(c + 1) * BL])
        in_maps.append(m)
    res = run_bass_kernel_spmd(nc, in_maps, list(range(NCORES)))
    out = np.concatenate([res.results[c]["y"] for c in range(NCORES)], axis=0)
    return out.astype(np.float32)

